# revision 32
# baseline (speedup 1.0000x reference)
"""GAT (graph attention) message-passing kernel for 8 Trainium2 NeuronCores.

Strategy (dst-sharded graph parallel, batched SWDGE gathers):
  - Host: add self loops; sort nodes by in-degree (desc) and deal them
    round-robin to the 8 cores so every core shares one degree profile.
    Node <-> (block b = loc//128, partition p = loc%128). All host work is
    index/permutation preprocessing.
  - Device phase 1 (per core, own 12500 nodes):
    ps = x_blk @ [W | W@a_src | W@a_dst] (W column-permuted to (c,h)-major)
    -> f16 table row [h_chw(128) | alpha_src(4) | pad(124)] (512B rows so
    dma_gather's 256B-multiple elem_size constraint holds), written to DRAM
    with a (p*ni+i)-interleaved row order so each 8-tile write is one
    full-width descriptor stream. alpha_dst stays SBUF-resident.
  - AllGather the f16 table (one collective, ~15us).
  - Phase 3: per-edge source rows are fetched with InstDMAGatherAnt
    (gpsimd.dma_gather): int16 indices force <=32767-row windows, so the
    global table is split into 4 sessions (2 owner cores each); every
    (block, session) gets max-per-partition padded columns pointing at a
    sentinel row whose alpha_src = -3e4 (=> p = 0, no mask tensor needed).
    Calls are <=1024 indices (SWDGE ring cap) on 2 alternating queues.
    Per column: p = exp(max(t, 0.2t)) (leaky-relu folded into max, one Exp),
    rhs = [p*h | p] via one f16 2x-mode broadcast multiply ((c,h)-major
    layout keeps the innermost axis packed), psum += I.T @ rhs on the PE.
    Epilogue: out = tanh(psum * (1/sum_p)) fused on ACT per head.

Softmax max-subtraction is dropped: logits are O(+-5) so exp is safe, and
softmax is shift-invariant; matches the reference to fp16 rounding error.
"""

import os
import numpy as np

import concourse.bacc as bacc
import concourse.bass as bass
import concourse.mybir as mybir
import concourse.tile as tile

NCORES = 8
NSESS = 4        # gather sessions (int16 index windows of 2 cores each)
P = 128          # partitions / block size / h channels
H = 4            # heads
C = 32           # channels per head
ROWB = 256       # table row f16 elements (512 bytes)
ACOL = P         # alpha_src column offset within a row
BG = 4           # dst blocks per gather group
WCALL = 8        # gather columns per dma_gather call (1024-desc ring cap)

F32 = mybir.dt.float32
F16 = mybir.dt.float16
I16 = mybir.dt.int16
I32 = mybir.dt.int32

SENT_VAL = -30000.0


def build_program(NLOC, NROWT, IN_DIM, Tbs, bias_nonzero):
    """One SPMD program for all cores.

    Tbs[b][s] = padded column count of dst block b, session s (uniform
    across cores). NROWT = per-core table rows (incl. sentinel).
    """
    NB = len(Tbs)
    last_nn = NLOC - (NB - 1) * P
    KT = IN_DIM // P
    WIN = 2 * NROWT                     # session window rows
    # column stream is (group, session, block)-major so each (group, session)
    # segment is one contiguous gather range
    NG = (NB + BG - 1) // BG
    colbase = {}
    t = 0
    for g in range(NG):
        for s in range(NSESS):
            for bi in range(min(BG, NB - g * BG)):
                colbase[(g * BG + bi, s)] = t
                t += Tbs[g * BG + bi][s]
    T_tot = t
    WGMAX = max(
        sum(Tbs[g * BG + bi][s]
            for bi in range(min(BG, NB - g * BG)))
        for g in range(NG) for s in range(NSESS))

    nc = bacc.Bacc("TRN2", target_bir_lowering=False, num_devices=NCORES,
                   num_swdge_queues=4)

    xT = nc.dram_tensor("xT", [IN_DIM, NLOC], F32, kind="ExternalInput")
    Wt = nc.dram_tensor("W", [IN_DIM, P], F32, kind="ExternalInput")
    asrc_b = nc.dram_tensor("asrc_b", [P, P], F32, kind="ExternalInput")
    adst_b = nc.dram_tensor("adst_b", [P, P], F32, kind="ExternalInput")
    bias_b = nc.dram_tensor("bias_b", [P, P], F32, kind="ExternalInput")
    ident_in = nc.dram_tensor("ident", [P, P], F16, kind="ExternalInput")
    gidx_in = nc.dram_tensor("gidx", [P, T_tot * 8], I16, kind="ExternalInput")

    out_d = nc.dram_tensor("out", [NB * P, P], F16, kind="ExternalOutput")

    TW = P + H        # compact table row width (132 f16)
    tbl_own = nc.dram_tensor("tbl_own", [NROWT, TW], F16, kind="Internal")
    tbl_cmp = nc.dram_tensor("tbl_cmp", [NCORES * NROWT, TW], F16,
                             kind="Internal", addr_space="Shared")
    tbl_full = nc.dram_tensor("tbl_full", [NCORES * NROWT, ROWB], F16,
                              kind="Internal")

    with tile.TileContext(nc) as tc:
        with tc.tile_pool(name="const", bufs=1) as cpool, \
             tc.tile_pool(name="p1", bufs=3) as p1pool, \
             tc.tile_pool(name="tr", bufs=3) as trpool, \
             tc.tile_pool(name="gat", bufs=6) as gpool, \
             tc.tile_pool(name="idx", bufs=6) as ipool, \
             tc.tile_pool(name="wrk", bufs=4) as wpool, \
             tc.tile_pool(name="epi", bufs=3) as epool, \
             tc.tile_pool(name="ps", bufs=1, space="PSUM") as ppool, \
             tc.tile_pool(name="ps1", bufs=2, space="PSUM") as p1ps:

            # ---- Phase 0: constants ----
            ident_sb = cpool.tile([P, P], F16)
            nc.sync.dma_start(out=ident_sb[:], in_=ident_in[:, :])
            asrc_sb = cpool.tile([P, P], F32)
            nc.sync.dma_start(out=asrc_sb[:], in_=asrc_b[:, :])
            adst_sb = cpool.tile([P, P], F32)
            nc.sync.dma_start(out=adst_sb[:], in_=adst_b[:, :])
            bias_sb = cpool.tile([P, P], F32)
            nc.sync.dma_start(out=bias_sb[:], in_=bias_b[:, :])

            # Extended weights [W | W@a_src | W@a_dst] per 128-row k-tile.
            # W columns are (c,h)-major; per-head reduce is over stride-H c.
            W_sb = cpool.tile([P, KT, P + 2 * H], F32)
            scr = cpool.tile([P, P], F32)
            for kt in range(KT):
                nc.sync.dma_start(out=W_sb[:, kt, 0:P],
                                  in_=Wt[kt * P:(kt + 1) * P, :])
            for kt in range(KT):
                for j, ab in ((0, asrc_sb), (1, adst_sb)):
                    nc.vector.tensor_tensor(
                        out=scr[:], in0=W_sb[:, kt, 0:P], in1=ab[:],
                        op=mybir.AluOpType.mult)
                    sv = scr[:].rearrange("p (c h) -> p c h", h=H)
                    for h in range(H):
                        nc.vector.tensor_reduce(
                            out=W_sb[:, kt, P + j * H + h:P + j * H + h + 1],
                            in_=sv[:, :, h],
                            axis=mybir.AxisListType.X,
                            op=mybir.AluOpType.add)

            adsb = cpool.tile([P, NB, H], F16)
            nc.vector.memset(adsb[:], 0.0)

            # ---- Phase 1: node table (own shard) ----
            XB = 8
            xt = None
            trow8 = None
            for b in range(NB):
                nn = P if b < NB - 1 else last_nn
                if b % XB == 0:
                    bw = min(XB * P, NLOC - b * P)
                    xt = p1pool.tile([P, KT, XB * P], F32, tag="xt")
                    for kt in range(KT):
                        nc.sync.dma_start(
                            out=xt[:, kt, :bw],
                            in_=xT[kt * P:(kt + 1) * P, b * P:b * P + bw])
                    trow8 = trpool.tile([P, XB, TW], F16, tag="trow8")
                o = (b % XB) * P
                ps1 = p1ps.tile([P, P + 2 * H], F32)
                for kt in range(KT):
                    nc.tensor.matmul(out=ps1[:nn, :],
                                     lhsT=xt[:, kt, o:o + nn],
                                     rhs=W_sb[:, kt, :],
                                     start=(kt == 0), stop=(kt == KT - 1))
                nc.scalar.activation(out=trow8[:nn, b % XB, 0:P + H],
                                     in_=ps1[:nn, 0:P + H],
                                     func=mybir.ActivationFunctionType.Copy)
                nc.vector.tensor_copy(out=adsb[:nn, b, :],
                                      in_=ps1[:nn, P + H:P + 2 * H])
                if b % XB == XB - 1 or b == NB - 1:
                    g0 = (b // XB) * XB
                    ni = b - g0 + 1
                    r0 = g0 * P
                    nc.sync.dma_start(
                        out=tbl_own[r0:r0 + ni * P, :],
                        in_=trow8[:, :ni, :])
            sent = cpool.tile([1, TW], F16)
            nc.vector.memset(sent[:], SENT_VAL)
            nc.sync.dma_start(out=tbl_own[NROWT - 1:NROWT, :], in_=sent[:])

            # ---- Phase 2: replicate node table ----
            # No engine barriers: the collective's RAW dependency on tbl_own
            # orders it after phase 1, gathers depend on tbl_full, and the
            # AllGather itself synchronizes across cores. Keeping engines
            # unblocked lets idx prefetch overlap the collective.
            nc.gpsimd.collective_compute(
                kind="AllGather",
                op=mybir.AluOpType.bypass,
                replica_groups=[list(range(NCORES))],
                ins=[tbl_own[:, :]],
                outs=[tbl_cmp[:, :]],
            )
            # expand compact rows into the 512B-pitch gather table, one DMA
            # per session window so early windows unblock early gathers
            for s in range(NSESS):
                nc.scalar.dma_start(
                    out=tbl_full[s * WIN:(s + 1) * WIN, 0:TW],
                    in_=tbl_cmp[s * WIN:(s + 1) * WIN, :])

            # ---- Phase 3: gather / attention / accumulate ----
            qsel = 0
            NG = (NB + BG - 1) // BG
            for g in range(NG):
                b0 = g * BG
                nblk = min(BG, NB - b0)
                pts = [ppool.tile([P, P + H], F32, name=f"pt{g}_{bi}",
                                  tag=f"pt{bi}")
                       for bi in range(nblk)]
                owide = epool.tile([P, BG, P], F16, tag="owide")
                for s in range(NSESS):
                    c0 = colbase[(b0, s)]
                    Wg = sum(Tbs[b0 + bi][s] for bi in range(nblk))
                    gt = gpool.tile([P, WGMAX, ROWB], F16, tag="gt")
                    idxt = ipool.tile([P, WGMAX * 8], I16, tag="idxt")
                    nc.sync.dma_start(out=idxt[:, :Wg * 8],
                                      in_=gidx_in[:, c0 * 8:(c0 + Wg) * 8])
                    for w0 in range(0, Wg, WCALL):
                        ww = min(WCALL, Wg - w0)
                        nc.gpsimd.dma_gather(
                            out_ap=gt[:, w0:w0 + ww, :],
                            in_ap=tbl_full[s * WIN:(s + 1) * WIN, :],
                            idxs_ap=idxt[:, w0 * 8:(w0 + ww) * 8],
                            num_idxs=ww * P,
                            num_idxs_reg=ww * P,
                            elem_size=ROWB,
                            queue_num=qsel % 4)
                        qsel += 1
                    # t = alpha_src + alpha_dst(block)
                    tsc = wpool.tile([P, WGMAX, H], F16, tag="tsc")
                    off = 0
                    for bi in range(nblk):
                        tb = Tbs[b0 + bi][s]
                        nc.vector.tensor_tensor(
                            out=tsc[:, off:off + tb, :],
                            in0=gt[:, off:off + tb, ACOL:ACOL + H],
                            in1=adsb[:, b0 + bi:b0 + bi + 1, :]
                                .broadcast_to([P, tb, H]),
                            op=mybir.AluOpType.add)
                        off += tb
                    # p = exp(max(t, 0.2t)) -> gt alpha slot
                    ts2 = wpool.tile([P, WGMAX, H], F16, tag="ts2")
                    nc.vector.tensor_scalar_mul(ts2[:, :Wg, :],
                                                tsc[:, :Wg, :], 0.2)
                    nc.vector.tensor_tensor(out=tsc[:, :Wg, :],
                                            in0=tsc[:, :Wg, :],
                                            in1=ts2[:, :Wg, :],
                                            op=mybir.AluOpType.max)
                    nc.scalar.activation(
                        out=gt[:, :Wg, ACOL:ACOL + H], in_=tsc[:, :Wg, :],
                        func=mybir.ActivationFunctionType.Exp)
                    # h *= p  ((c,h)-major keeps innermost packed: 2x mode)
                    nc.vector.tensor_tensor(
                        out=gt[:, :Wg, 0:P].rearrange(
                            "p t (c h) -> p t c h", h=H),
                        in0=gt[:, :Wg, 0:P].rearrange(
                            "p t (c h) -> p t c h", h=H),
                        in1=gt[:, :Wg, ACOL:ACOL + H][:, :, None, :]
                            .broadcast_to([P, Wg, C, H]),
                        op=mybir.AluOpType.mult)
                    off = 0
                    for bi in range(nblk):
                        tb = Tbs[b0 + bi][s]
                        for j in range(tb):
                            nc.tensor.matmul(
                                out=pts[bi][:],
                                lhsT=ident_sb[:],
                                rhs=gt[:, off + j, 0:P + H],
                                start=(s == 0 and j == 0),
                                stop=(s == NSESS - 1 and j == tb - 1))
                        off += tb
                # epilogue: out = tanh(psum * rcp) per head (bias is zero)
                for bi in range(nblk):
                    nn = P if b0 + bi < NB - 1 else last_nn
                    pt = pts[bi]
                    rcp = epool.tile([P, H], F32, tag="rcp")
                    nc.vector.reciprocal(rcp[:nn, :], pt[:nn, P:P + H])
                    ptv = pt[:nn, 0:P].rearrange("p (c h) -> p c h", h=H)
                    if bias_nonzero:
                        osb = epool.tile([P, P], F32, tag="osb")
                        nc.vector.tensor_tensor(
                            out=osb[:nn, :].rearrange("p (c h) -> p c h", h=H),
                            in0=ptv,
                            in1=rcp[:nn, None, :].broadcast_to([nn, C, H]),
                            op=mybir.AluOpType.mult)
                        nc.vector.tensor_tensor(
                            out=osb[:nn, :], in0=osb[:nn, :],
                            in1=bias_sb[:nn, :], op=mybir.AluOpType.add)
                        nc.scalar.activation(
                            out=owide[:nn, bi, :], in_=osb[:nn, :],
                            func=mybir.ActivationFunctionType.Tanh)
                    else:
                        ov = owide[:nn, bi, :].rearrange("p (c h) -> p c h",
                                                         h=H)
                        for h in range(H):
                            nc.scalar.activation(
                                out=ov[:, :, h], in_=ptv[:, :, h],
                                func=mybir.ActivationFunctionType.Tanh,
                                scale=rcp[:nn, h:h + 1])
                # dispatch from ACT's queue: the write follows its producing
                # tanh in program order, so the SEQ-held wait is ~zero and
                # SP's queue stays free for the next group's idx loads
                nc.scalar.dma_start(
                    out=out_d[b0 * P:(b0 + nblk) * P, :],
                    in_=owide[:, :nblk, :])
    nc.finalize()
    return nc


def host_prepare(x, W, a_src, a_dst, bias, edge_index):
    """Index/permutation preprocessing only - no float math on node data."""
    N, IN_DIM = x.shape
    NLOC = N // NCORES
    NB = (NLOC + P - 1) // P
    NBGFULL = NB // 8                       # full 8-tile write groups
    NROWT = NB * P + 1                      # padded rows + sentinel
    SENT_REL = NROWT - 1 - 0                # sentinel of first core in window

    ei = np.asarray(edge_index)
    loops = np.arange(N, dtype=np.int64)
    src = np.concatenate([loops, ei[0].astype(np.int64)])
    dst = np.concatenate([loops, ei[1].astype(np.int64)])
    del loops

    deg = np.bincount(dst, minlength=N)
    rank = np.argsort(-deg, kind="stable")      # rank r -> node
    rank_inv = np.empty(N, dtype=np.int64)      # node -> rank
    rank_inv[rank] = np.arange(N)

    # Greedy owner assignment within each rank-octet: each octet still gives
    # one node per core (degree profiles stay uniform), but the pairing is
    # chosen to balance every dst node's in-edges across the 4 gather
    # sessions (session = owner//2), shrinking per-(block,session) padding.
    csr_order = np.argsort(src, kind="stable")
    dst_sorted = dst[csr_order]
    starts = np.zeros(N + 1, dtype=np.int64)
    starts[1:] = np.cumsum(np.bincount(src, minlength=N))
    cnt4 = np.zeros((N, NSESS), dtype=np.int32)
    choice = np.full(N, -1, dtype=np.int8)
    big = 1 << 30
    for k in range(N // NCORES):
        octet = rank[k * NCORES:(k + 1) * NCORES]
        od = starts[octet + 1] - starts[octet]
        cap = [2] * NSESS
        for n in octet[np.argsort(-od)]:
            ds = dst_sorted[starts[n]:starts[n + 1]]
            costs = (3.0 ** cnt4[ds]).sum(axis=0) if len(ds) \
                else np.zeros(NSESS)
            q = int(np.argmin(
                [c + (0 if cap[i] > 0 else big)
                 for i, c in enumerate(costs)]))
            cap[q] -= 1
            choice[n] = q
            if len(ds):
                cnt4[ds, q] += 1
    # refinement: per-octet pairwise session swaps that lower the convex cost
    for _ in range(2):
        for k in range(N // NCORES):
            octet = rank[k * NCORES:(k + 1) * NCORES]
            for n in octet:
                ds = dst_sorted[starts[n]:starts[n + 1]]
                if not len(ds):
                    continue
                q0 = int(choice[n])
                cnt4[ds, q0] -= 1
                c = (3.0 ** cnt4[ds]).sum(axis=0)
                best, bq, bpart = c[q0], q0, -1
                for q in range(NSESS):
                    if q == q0:
                        continue
                    for m in octet:
                        if choice[m] == q and m != n:
                            dm = dst_sorted[starts[m]:starts[m + 1]]
                            cnt4[dm, q] -= 1
                            e = 3.0 ** cnt4[dm]
                            cn = c[q] + e[:, q0].sum() - e[:, q].sum()
                            cnt4[dm, q] += 1
                            if cn < best - 1e-9:
                                best, bq, bpart = cn, q, m
                            break
                if bq != q0:
                    m = bpart
                    dm = dst_sorted[starts[m]:starts[m + 1]]
                    cnt4[dm, bq] -= 1
                    cnt4[dm, q0] += 1
                    choice[m] = q0
                    choice[n] = bq
                    cnt4[ds, bq] += 1
                else:
                    cnt4[ds, q0] += 1
    owner = np.empty(N, dtype=np.int64)
    for k in range(N // NCORES):
        octet = rank[k * NCORES:(k + 1) * NCORES]
        used = [0] * NSESS
        for n in octet:
            q = int(choice[n])
            owner[n] = 2 * q + used[q]
            used[q] += 1
    loc = rank_inv // NCORES
    blk = loc // P
    part = loc % P

    # table row (interleaved so phase-1 writes are full-width streams)
    grp = blk // 8
    i_in_g = blk - grp * 8
    ni = np.where(grp < NBGFULL, 8, NB - NBGFULL * 8)
    tblrow = grp * (8 * P) + part * ni + i_in_g      # local row
    sess_of = owner // 2
    rel_idx = (owner % 2) * NROWT + tblrow          # idx within session window

    # per-edge coordinates on the dst side
    e_core = owner[dst]
    e_blk = blk[dst]
    e_part = part[dst]
    e_sess = sess_of[src]

    # per (block, session) padded widths: max slot count over (core, partition)
    key = ((e_blk * NSESS + e_sess) * NCORES + e_core) * P + e_part
    cnt = np.bincount(key, minlength=NB * NSESS * NCORES * P)
    cnt = cnt.reshape(NB, NSESS, NCORES * P)
    Tbs = np.maximum(cnt.max(axis=2), 1)            # [NB, NSESS]

    # (group, session, block)-major column layout (must match build_program)
    colbase = np.zeros((NB, NSESS), dtype=np.int64)
    t = 0
    NGB = (NB + BG - 1) // BG
    for g in range(NGB):
        for s in range(NSESS):
            for bi in range(min(BG, NB - g * BG)):
                colbase[g * BG + bi, s] = t
                t += int(Tbs[g * BG + bi, s])
    T_tot = int(Tbs.sum())

    # slot within (core, block, part, sess): order of appearance
    order = np.argsort(key, kind="stable")
    ks = key[order]
    slot = np.arange(len(ks)) - np.concatenate(
        ([0], np.cumsum(np.bincount(ks, minlength=key.max() + 1))[:-1]))[ks]
    e_slot = np.empty(len(key), dtype=np.int64)
    e_slot[order] = slot

    e_col = colbase[e_blk, e_sess] + e_slot         # global column
    e_rel = rel_idx[src].astype(np.int16)

    ident = np.eye(P, dtype=np.float16)
    # (c,h)-major column permutation: perm_ch[c*H+h] = h*C+c
    cc, hh = np.meshgrid(np.arange(C), np.arange(H), indexing="ij")
    perm_ch = (hh * C + cc).ravel()
    Wf = np.ascontiguousarray(np.asarray(W, np.float32)[:, perm_ch])
    a_src_p = np.asarray(a_src, np.float32).ravel()[perm_ch]
    a_dst_p = np.asarray(a_dst, np.float32).ravel()[perm_ch]
    bias_p = np.asarray(bias, np.float32).ravel()[perm_ch]
    asrc_bt = np.tile(a_src_p.reshape(1, -1), (P, 1))
    adst_bt = np.tile(a_dst_p.reshape(1, -1), (P, 1))
    bias_bt = np.tile(bias_p.reshape(1, -1), (P, 1))
    xf = np.asarray(x, np.float32)

    in_maps = []
    for k in range(NCORES):
        sel = e_core == k
        col_k = e_col[sel]
        part_k = e_part[sel]
        rel_k = e_rel[sel]

        gidx = np.full((P, T_tot * 8), SENT_REL, dtype=np.int16)
        fcol = col_k * 8 + part_k // 16
        frow = part_k % 16
        for gg in range(8):
            gidx[16 * gg + frow, fcol] = rel_k

        selk = np.flatnonzero(owner == k)
        own_nodes = np.empty(NLOC, dtype=np.int64)
        own_nodes[loc[selk]] = selk
        xT_k = np.ascontiguousarray(xf[own_nodes].T)
        in_maps.append({
            "xT": xT_k, "W": Wf, "asrc_b": asrc_bt, "adst_b": adst_bt,
            "bias_b": bias_bt, "ident": ident, "gidx": gidx,
        })

    cfg = dict(NLOC=NLOC, NROWT=NROWT, IN_DIM=IN_DIM,
               bias_nonzero=bool(np.any(np.asarray(bias))))
    meta = dict(Tbs=[[int(v) for v in row] for row in Tbs],
                owner=owner, loc=loc, perm_ch=perm_ch)
    return cfg, meta, in_maps


def assemble_output(results, N, owner, loc, perm_ch):
    NLOC = N // NCORES
    NB = (NLOC + P - 1) // P
    ll = np.arange(NLOC)
    b = ll // P
    p = ll % P
    g4 = b // BG
    nbg = np.where(g4 < NB // BG, BG, NB - (NB // BG) * BG)
    # out row written as [p, block-in-group, ch] -> row b0*P + p*nbg + i
    row = g4 * (BG * P) + p * nbg + (b - g4 * BG)

    inv_perm = np.empty(P, dtype=np.int64)
    inv_perm[perm_ch] = np.arange(P)                # old h*C+c <- new c*H+h

    out = np.empty((N, P), np.float32)
    for k in range(NCORES):
        selk = np.flatnonzero(owner == k)
        own_nodes = np.empty(NLOC, dtype=np.int64)
        own_nodes[loc[selk]] = selk
        res = results[k]["out"].astype(np.float32)
        out[own_nodes] = res[row][:, inv_perm]
    return out


LAST_RESULTS = None


def kernel(x, W, a_src, a_dst, bias, edge_index):
    global LAST_RESULTS
    from concourse.bass_utils import run_bass_kernel_spmd

    cfg, meta, in_maps = host_prepare(x, W, a_src, a_dst, bias, edge_index)
    nc = build_program(cfg["NLOC"], cfg["NROWT"], cfg["IN_DIM"],
                       meta["Tbs"], cfg["bias_nonzero"])
    res = run_bass_kernel_spmd(
        nc, in_maps, core_ids=list(range(NCORES)),
        trace=os.environ.get("GAT_TRACE", "0") == "1")
    LAST_RESULTS = res
    return assemble_output(res.results, x.shape[0], meta["owner"],
                           meta["loc"], meta["perm_ch"])


# revision 33
# speedup vs baseline: 1.0004x; 1.0004x over previous
"""GAT (graph attention) message-passing kernel for 8 Trainium2 NeuronCores.

Strategy (dst-sharded graph parallel, batched SWDGE gathers):
  - Host: add self loops; sort nodes by in-degree (desc) and deal them
    round-robin to the 8 cores so every core shares one degree profile.
    Node <-> (block b = loc//128, partition p = loc%128). All host work is
    index/permutation preprocessing.
  - Device phase 1 (per core, own 12500 nodes):
    ps = x_blk @ [W | W@a_src | W@a_dst] (W column-permuted to (c,h)-major)
    -> f16 table row [h_chw(128) | alpha_src(4) | pad(124)] (512B rows so
    dma_gather's 256B-multiple elem_size constraint holds), written to DRAM
    with a (p*ni+i)-interleaved row order so each 8-tile write is one
    full-width descriptor stream. alpha_dst stays SBUF-resident.
  - AllGather the f16 table (one collective, ~15us).
  - Phase 3: per-edge source rows are fetched with InstDMAGatherAnt
    (gpsimd.dma_gather): int16 indices force <=32767-row windows, so the
    global table is split into 4 sessions (2 owner cores each); every
    (block, session) gets max-per-partition padded columns pointing at a
    sentinel row whose alpha_src = -3e4 (=> p = 0, no mask tensor needed).
    Calls are <=1024 indices (SWDGE ring cap) on 2 alternating queues.
    Per column: p = exp(max(t, 0.2t)) (leaky-relu folded into max, one Exp),
    rhs = [p*h | p] via one f16 2x-mode broadcast multiply ((c,h)-major
    layout keeps the innermost axis packed), psum += I.T @ rhs on the PE.
    Epilogue: out = tanh(psum * (1/sum_p)) fused on ACT per head.

Softmax max-subtraction is dropped: logits are O(+-5) so exp is safe, and
softmax is shift-invariant; matches the reference to fp16 rounding error.
"""

import os
import numpy as np

import concourse.bacc as bacc
import concourse.bass as bass
import concourse.mybir as mybir
import concourse.tile as tile

NCORES = 8
NSESS = 4        # gather sessions (int16 index windows of 2 cores each)
P = 128          # partitions / block size / h channels
H = 4            # heads
C = 32           # channels per head
ROWB = 256       # table row f16 elements (512 bytes)
ACOL = P         # alpha_src column offset within a row
BG = 4           # dst blocks per gather group
WCALL = 8        # gather columns per dma_gather call (1024-desc ring cap)

F32 = mybir.dt.float32
F16 = mybir.dt.float16
I16 = mybir.dt.int16
I32 = mybir.dt.int32

SENT_VAL = -30000.0


def build_program(NLOC, NROWT, IN_DIM, Tbs, bias_nonzero):
    """One SPMD program for all cores.

    Tbs[b][s] = padded column count of dst block b, session s (uniform
    across cores). NROWT = per-core table rows (incl. sentinel).
    """
    NB = len(Tbs)
    last_nn = NLOC - (NB - 1) * P
    KT = IN_DIM // P
    WIN = 2 * NROWT                     # session window rows
    # column stream is (group, session, block)-major so each (group, session)
    # segment is one contiguous gather range
    NG = (NB + BG - 1) // BG
    colbase = {}
    t = 0
    for g in range(NG):
        for s in range(NSESS):
            for bi in range(min(BG, NB - g * BG)):
                colbase[(g * BG + bi, s)] = t
                t += Tbs[g * BG + bi][s]
    T_tot = t
    WGMAX = max(
        sum(Tbs[g * BG + bi][s]
            for bi in range(min(BG, NB - g * BG)))
        for g in range(NG) for s in range(NSESS))

    nc = bacc.Bacc("TRN2", target_bir_lowering=False, num_devices=NCORES,
                   num_swdge_queues=2)

    xT = nc.dram_tensor("xT", [IN_DIM, NLOC], F32, kind="ExternalInput")
    Wt = nc.dram_tensor("W", [IN_DIM, P], F32, kind="ExternalInput")
    asrc_b = nc.dram_tensor("asrc_b", [P, P], F32, kind="ExternalInput")
    adst_b = nc.dram_tensor("adst_b", [P, P], F32, kind="ExternalInput")
    bias_b = nc.dram_tensor("bias_b", [P, P], F32, kind="ExternalInput")
    ident_in = nc.dram_tensor("ident", [P, P], F16, kind="ExternalInput")
    gidx_in = nc.dram_tensor("gidx", [P, T_tot * 8], I16, kind="ExternalInput")

    out_d = nc.dram_tensor("out", [NB * P, P], F16, kind="ExternalOutput")

    TW = P + H        # compact table row width (132 f16)
    tbl_own = nc.dram_tensor("tbl_own", [NROWT, TW], F16, kind="Internal")
    tbl_cmp = nc.dram_tensor("tbl_cmp", [NCORES * NROWT, TW], F16,
                             kind="Internal", addr_space="Shared")
    tbl_full = nc.dram_tensor("tbl_full", [NCORES * NROWT, ROWB], F16,
                              kind="Internal")

    with tile.TileContext(nc) as tc:
        with tc.tile_pool(name="const", bufs=1) as cpool, \
             tc.tile_pool(name="p1", bufs=3) as p1pool, \
             tc.tile_pool(name="tr", bufs=3) as trpool, \
             tc.tile_pool(name="gat", bufs=6) as gpool, \
             tc.tile_pool(name="idx", bufs=6) as ipool, \
             tc.tile_pool(name="wrk", bufs=4) as wpool, \
             tc.tile_pool(name="epi", bufs=3) as epool, \
             tc.tile_pool(name="ps", bufs=1, space="PSUM") as ppool, \
             tc.tile_pool(name="ps1", bufs=2, space="PSUM") as p1ps:

            # ---- Phase 0: constants ----
            ident_sb = cpool.tile([P, P], F16)
            nc.sync.dma_start(out=ident_sb[:], in_=ident_in[:, :])
            asrc_sb = cpool.tile([P, P], F32)
            nc.sync.dma_start(out=asrc_sb[:], in_=asrc_b[:, :])
            adst_sb = cpool.tile([P, P], F32)
            nc.sync.dma_start(out=adst_sb[:], in_=adst_b[:, :])
            bias_sb = cpool.tile([P, P], F32)
            nc.sync.dma_start(out=bias_sb[:], in_=bias_b[:, :])

            # Extended weights [W | W@a_src | W@a_dst] per 128-row k-tile.
            # W columns are (c,h)-major; per-head reduce is over stride-H c.
            W_sb = cpool.tile([P, KT, P + 2 * H], F32)
            scr = cpool.tile([P, P], F32)
            for kt in range(KT):
                nc.sync.dma_start(out=W_sb[:, kt, 0:P],
                                  in_=Wt[kt * P:(kt + 1) * P, :])
            for kt in range(KT):
                for j, ab in ((0, asrc_sb), (1, adst_sb)):
                    nc.vector.tensor_tensor(
                        out=scr[:], in0=W_sb[:, kt, 0:P], in1=ab[:],
                        op=mybir.AluOpType.mult)
                    sv = scr[:].rearrange("p (c h) -> p c h", h=H)
                    for h in range(H):
                        nc.vector.tensor_reduce(
                            out=W_sb[:, kt, P + j * H + h:P + j * H + h + 1],
                            in_=sv[:, :, h],
                            axis=mybir.AxisListType.X,
                            op=mybir.AluOpType.add)

            adsb = cpool.tile([P, NB, H], F16)
            nc.vector.memset(adsb[:], 0.0)

            # ---- Phase 1: node table (own shard) ----
            XB = 8
            xt = None
            trow8 = None
            for b in range(NB):
                nn = P if b < NB - 1 else last_nn
                if b % XB == 0:
                    bw = min(XB * P, NLOC - b * P)
                    xt = p1pool.tile([P, KT, XB * P], F32, tag="xt")
                    for kt in range(KT):
                        nc.sync.dma_start(
                            out=xt[:, kt, :bw],
                            in_=xT[kt * P:(kt + 1) * P, b * P:b * P + bw])
                    trow8 = trpool.tile([P, XB, TW], F16, tag="trow8")
                o = (b % XB) * P
                ps1 = p1ps.tile([P, P + 2 * H], F32)
                for kt in range(KT):
                    nc.tensor.matmul(out=ps1[:nn, :],
                                     lhsT=xt[:, kt, o:o + nn],
                                     rhs=W_sb[:, kt, :],
                                     start=(kt == 0), stop=(kt == KT - 1))
                nc.scalar.activation(out=trow8[:nn, b % XB, 0:P + H],
                                     in_=ps1[:nn, 0:P + H],
                                     func=mybir.ActivationFunctionType.Copy)
                nc.vector.tensor_copy(out=adsb[:nn, b, :],
                                      in_=ps1[:nn, P + H:P + 2 * H])
                if b % XB == XB - 1 or b == NB - 1:
                    g0 = (b // XB) * XB
                    ni = b - g0 + 1
                    r0 = g0 * P
                    nc.sync.dma_start(
                        out=tbl_own[r0:r0 + ni * P, :],
                        in_=trow8[:, :ni, :])
            sent = cpool.tile([1, TW], F16)
            nc.vector.memset(sent[:], SENT_VAL)
            nc.sync.dma_start(out=tbl_own[NROWT - 1:NROWT, :], in_=sent[:])

            # ---- Phase 2: replicate node table ----
            # No engine barriers: the collective's RAW dependency on tbl_own
            # orders it after phase 1, gathers depend on tbl_full, and the
            # AllGather itself synchronizes across cores. Keeping engines
            # unblocked lets idx prefetch overlap the collective.
            nc.gpsimd.collective_compute(
                kind="AllGather",
                op=mybir.AluOpType.bypass,
                replica_groups=[list(range(NCORES))],
                ins=[tbl_own[:, :]],
                outs=[tbl_cmp[:, :]],
            )
            # expand compact rows into the 512B-pitch gather table, one DMA
            # per session window so early windows unblock early gathers
            for s in range(NSESS):
                nc.scalar.dma_start(
                    out=tbl_full[s * WIN:(s + 1) * WIN, 0:TW],
                    in_=tbl_cmp[s * WIN:(s + 1) * WIN, :])

            # ---- Phase 3: gather / attention / accumulate ----
            qsel = 0
            NG = (NB + BG - 1) // BG
            for g in range(NG):
                b0 = g * BG
                nblk = min(BG, NB - b0)
                pts = [ppool.tile([P, P + H], F32, name=f"pt{g}_{bi}",
                                  tag=f"pt{bi}")
                       for bi in range(nblk)]
                owide = epool.tile([P, BG, P], F16, tag="owide")
                for s in range(NSESS):
                    c0 = colbase[(b0, s)]
                    Wg = sum(Tbs[b0 + bi][s] for bi in range(nblk))
                    gt = gpool.tile([P, WGMAX, ROWB], F16, tag="gt")
                    idxt = ipool.tile([P, WGMAX * 8], I16, tag="idxt")
                    nc.sync.dma_start(out=idxt[:, :Wg * 8],
                                      in_=gidx_in[:, c0 * 8:(c0 + Wg) * 8])
                    for w0 in range(0, Wg, WCALL):
                        ww = min(WCALL, Wg - w0)
                        nc.gpsimd.dma_gather(
                            out_ap=gt[:, w0:w0 + ww, :],
                            in_ap=tbl_full[s * WIN:(s + 1) * WIN, :],
                            idxs_ap=idxt[:, w0 * 8:(w0 + ww) * 8],
                            num_idxs=ww * P,
                            num_idxs_reg=ww * P,
                            elem_size=ROWB,
                            queue_num=qsel % 2)
                        qsel += 1
                    # t = alpha_src + alpha_dst(block)
                    tsc = wpool.tile([P, WGMAX, H], F16, tag="tsc")
                    off = 0
                    for bi in range(nblk):
                        tb = Tbs[b0 + bi][s]
                        nc.vector.tensor_tensor(
                            out=tsc[:, off:off + tb, :],
                            in0=gt[:, off:off + tb, ACOL:ACOL + H],
                            in1=adsb[:, b0 + bi:b0 + bi + 1, :]
                                .broadcast_to([P, tb, H]),
                            op=mybir.AluOpType.add)
                        off += tb
                    # p = exp(max(t, 0.2t)) -> gt alpha slot
                    ts2 = wpool.tile([P, WGMAX, H], F16, tag="ts2")
                    nc.vector.tensor_scalar_mul(ts2[:, :Wg, :],
                                                tsc[:, :Wg, :], 0.2)
                    nc.vector.tensor_tensor(out=tsc[:, :Wg, :],
                                            in0=tsc[:, :Wg, :],
                                            in1=ts2[:, :Wg, :],
                                            op=mybir.AluOpType.max)
                    nc.scalar.activation(
                        out=gt[:, :Wg, ACOL:ACOL + H], in_=tsc[:, :Wg, :],
                        func=mybir.ActivationFunctionType.Exp)
                    # h *= p  ((c,h)-major keeps innermost packed: 2x mode)
                    nc.vector.tensor_tensor(
                        out=gt[:, :Wg, 0:P].rearrange(
                            "p t (c h) -> p t c h", h=H),
                        in0=gt[:, :Wg, 0:P].rearrange(
                            "p t (c h) -> p t c h", h=H),
                        in1=gt[:, :Wg, ACOL:ACOL + H][:, :, None, :]
                            .broadcast_to([P, Wg, C, H]),
                        op=mybir.AluOpType.mult)
                    off = 0
                    for bi in range(nblk):
                        tb = Tbs[b0 + bi][s]
                        for j in range(tb):
                            nc.tensor.matmul(
                                out=pts[bi][:],
                                lhsT=ident_sb[:],
                                rhs=gt[:, off + j, 0:P + H],
                                start=(s == 0 and j == 0),
                                stop=(s == NSESS - 1 and j == tb - 1))
                        off += tb
                # epilogue: out = tanh(psum * rcp) per head (bias is zero)
                for bi in range(nblk):
                    nn = P if b0 + bi < NB - 1 else last_nn
                    pt = pts[bi]
                    rcp = epool.tile([P, H], F32, tag="rcp")
                    nc.vector.reciprocal(rcp[:nn, :], pt[:nn, P:P + H])
                    ptv = pt[:nn, 0:P].rearrange("p (c h) -> p c h", h=H)
                    if bias_nonzero:
                        osb = epool.tile([P, P], F32, tag="osb")
                        nc.vector.tensor_tensor(
                            out=osb[:nn, :].rearrange("p (c h) -> p c h", h=H),
                            in0=ptv,
                            in1=rcp[:nn, None, :].broadcast_to([nn, C, H]),
                            op=mybir.AluOpType.mult)
                        nc.vector.tensor_tensor(
                            out=osb[:nn, :], in0=osb[:nn, :],
                            in1=bias_sb[:nn, :], op=mybir.AluOpType.add)
                        nc.scalar.activation(
                            out=owide[:nn, bi, :], in_=osb[:nn, :],
                            func=mybir.ActivationFunctionType.Tanh)
                    else:
                        ov = owide[:nn, bi, :].rearrange("p (c h) -> p c h",
                                                         h=H)
                        for h in range(H):
                            nc.scalar.activation(
                                out=ov[:, :, h], in_=ptv[:, :, h],
                                func=mybir.ActivationFunctionType.Tanh,
                                scale=rcp[:nn, h:h + 1])
                nc.sync.dma_start(
                    out=out_d[b0 * P:(b0 + nblk) * P, :],
                    in_=owide[:, :nblk, :])
    nc.finalize()
    return nc


def host_prepare(x, W, a_src, a_dst, bias, edge_index):
    """Index/permutation preprocessing only - no float math on node data."""
    N, IN_DIM = x.shape
    NLOC = N // NCORES
    NB = (NLOC + P - 1) // P
    NBGFULL = NB // 8                       # full 8-tile write groups
    NROWT = NB * P + 1                      # padded rows + sentinel
    SENT_REL = NROWT - 1 - 0                # sentinel of first core in window

    ei = np.asarray(edge_index)
    loops = np.arange(N, dtype=np.int64)
    src = np.concatenate([loops, ei[0].astype(np.int64)])
    dst = np.concatenate([loops, ei[1].astype(np.int64)])
    del loops

    deg = np.bincount(dst, minlength=N)
    rank = np.argsort(-deg, kind="stable")      # rank r -> node
    rank_inv = np.empty(N, dtype=np.int64)      # node -> rank
    rank_inv[rank] = np.arange(N)

    # Greedy owner assignment within each rank-octet: each octet still gives
    # one node per core (degree profiles stay uniform), but the pairing is
    # chosen to balance every dst node's in-edges across the 4 gather
    # sessions (session = owner//2), shrinking per-(block,session) padding.
    csr_order = np.argsort(src, kind="stable")
    dst_sorted = dst[csr_order]
    starts = np.zeros(N + 1, dtype=np.int64)
    starts[1:] = np.cumsum(np.bincount(src, minlength=N))
    cnt4 = np.zeros((N, NSESS), dtype=np.int32)
    choice = np.full(N, -1, dtype=np.int8)
    big = 1 << 30
    for k in range(N // NCORES):
        octet = rank[k * NCORES:(k + 1) * NCORES]
        od = starts[octet + 1] - starts[octet]
        cap = [2] * NSESS
        for n in octet[np.argsort(-od)]:
            ds = dst_sorted[starts[n]:starts[n + 1]]
            costs = (3.0 ** cnt4[ds]).sum(axis=0) if len(ds) \
                else np.zeros(NSESS)
            q = int(np.argmin(
                [c + (0 if cap[i] > 0 else big)
                 for i, c in enumerate(costs)]))
            cap[q] -= 1
            choice[n] = q
            if len(ds):
                cnt4[ds, q] += 1
    # refinement: per-octet pairwise session swaps that lower the convex cost
    for _ in range(2):
        for k in range(N // NCORES):
            octet = rank[k * NCORES:(k + 1) * NCORES]
            for n in octet:
                ds = dst_sorted[starts[n]:starts[n + 1]]
                if not len(ds):
                    continue
                q0 = int(choice[n])
                cnt4[ds, q0] -= 1
                c = (3.0 ** cnt4[ds]).sum(axis=0)
                best, bq, bpart = c[q0], q0, -1
                for q in range(NSESS):
                    if q == q0:
                        continue
                    for m in octet:
                        if choice[m] == q and m != n:
                            dm = dst_sorted[starts[m]:starts[m + 1]]
                            cnt4[dm, q] -= 1
                            e = 3.0 ** cnt4[dm]
                            cn = c[q] + e[:, q0].sum() - e[:, q].sum()
                            cnt4[dm, q] += 1
                            if cn < best - 1e-9:
                                best, bq, bpart = cn, q, m
                            break
                if bq != q0:
                    m = bpart
                    dm = dst_sorted[starts[m]:starts[m + 1]]
                    cnt4[dm, bq] -= 1
                    cnt4[dm, q0] += 1
                    choice[m] = q0
                    choice[n] = bq
                    cnt4[ds, bq] += 1
                else:
                    cnt4[ds, q0] += 1
    owner = np.empty(N, dtype=np.int64)
    for k in range(N // NCORES):
        octet = rank[k * NCORES:(k + 1) * NCORES]
        used = [0] * NSESS
        for n in octet:
            q = int(choice[n])
            owner[n] = 2 * q + used[q]
            used[q] += 1
    loc = rank_inv // NCORES
    blk = loc // P
    part = loc % P

    # table row (interleaved so phase-1 writes are full-width streams)
    grp = blk // 8
    i_in_g = blk - grp * 8
    ni = np.where(grp < NBGFULL, 8, NB - NBGFULL * 8)
    tblrow = grp * (8 * P) + part * ni + i_in_g      # local row
    sess_of = owner // 2
    rel_idx = (owner % 2) * NROWT + tblrow          # idx within session window

    # per-edge coordinates on the dst side
    e_core = owner[dst]
    e_blk = blk[dst]
    e_part = part[dst]
    e_sess = sess_of[src]

    # per (block, session) padded widths: max slot count over (core, partition)
    key = ((e_blk * NSESS + e_sess) * NCORES + e_core) * P + e_part
    cnt = np.bincount(key, minlength=NB * NSESS * NCORES * P)
    cnt = cnt.reshape(NB, NSESS, NCORES * P)
    Tbs = np.maximum(cnt.max(axis=2), 1)            # [NB, NSESS]

    # (group, session, block)-major column layout (must match build_program)
    colbase = np.zeros((NB, NSESS), dtype=np.int64)
    t = 0
    NGB = (NB + BG - 1) // BG
    for g in range(NGB):
        for s in range(NSESS):
            for bi in range(min(BG, NB - g * BG)):
                colbase[g * BG + bi, s] = t
                t += int(Tbs[g * BG + bi, s])
    T_tot = int(Tbs.sum())

    # slot within (core, block, part, sess): order of appearance
    order = np.argsort(key, kind="stable")
    ks = key[order]
    slot = np.arange(len(ks)) - np.concatenate(
        ([0], np.cumsum(np.bincount(ks, minlength=key.max() + 1))[:-1]))[ks]
    e_slot = np.empty(len(key), dtype=np.int64)
    e_slot[order] = slot

    e_col = colbase[e_blk, e_sess] + e_slot         # global column
    e_rel = rel_idx[src].astype(np.int16)

    ident = np.eye(P, dtype=np.float16)
    # (c,h)-major column permutation: perm_ch[c*H+h] = h*C+c
    cc, hh = np.meshgrid(np.arange(C), np.arange(H), indexing="ij")
    perm_ch = (hh * C + cc).ravel()
    Wf = np.ascontiguousarray(np.asarray(W, np.float32)[:, perm_ch])
    a_src_p = np.asarray(a_src, np.float32).ravel()[perm_ch]
    a_dst_p = np.asarray(a_dst, np.float32).ravel()[perm_ch]
    bias_p = np.asarray(bias, np.float32).ravel()[perm_ch]
    asrc_bt = np.tile(a_src_p.reshape(1, -1), (P, 1))
    adst_bt = np.tile(a_dst_p.reshape(1, -1), (P, 1))
    bias_bt = np.tile(bias_p.reshape(1, -1), (P, 1))
    xf = np.asarray(x, np.float32)

    in_maps = []
    for k in range(NCORES):
        sel = e_core == k
        col_k = e_col[sel]
        part_k = e_part[sel]
        rel_k = e_rel[sel]

        gidx = np.full((P, T_tot * 8), SENT_REL, dtype=np.int16)
        fcol = col_k * 8 + part_k // 16
        frow = part_k % 16
        for gg in range(8):
            gidx[16 * gg + frow, fcol] = rel_k

        selk = np.flatnonzero(owner == k)
        own_nodes = np.empty(NLOC, dtype=np.int64)
        own_nodes[loc[selk]] = selk
        xT_k = np.ascontiguousarray(xf[own_nodes].T)
        in_maps.append({
            "xT": xT_k, "W": Wf, "asrc_b": asrc_bt, "adst_b": adst_bt,
            "bias_b": bias_bt, "ident": ident, "gidx": gidx,
        })

    cfg = dict(NLOC=NLOC, NROWT=NROWT, IN_DIM=IN_DIM,
               bias_nonzero=bool(np.any(np.asarray(bias))))
    meta = dict(Tbs=[[int(v) for v in row] for row in Tbs],
                owner=owner, loc=loc, perm_ch=perm_ch)
    return cfg, meta, in_maps


def assemble_output(results, N, owner, loc, perm_ch):
    NLOC = N // NCORES
    NB = (NLOC + P - 1) // P
    ll = np.arange(NLOC)
    b = ll // P
    p = ll % P
    g4 = b // BG
    nbg = np.where(g4 < NB // BG, BG, NB - (NB // BG) * BG)
    # out row written as [p, block-in-group, ch] -> row b0*P + p*nbg + i
    row = g4 * (BG * P) + p * nbg + (b - g4 * BG)

    inv_perm = np.empty(P, dtype=np.int64)
    inv_perm[perm_ch] = np.arange(P)                # old h*C+c <- new c*H+h

    out = np.empty((N, P), np.float32)
    for k in range(NCORES):
        selk = np.flatnonzero(owner == k)
        own_nodes = np.empty(NLOC, dtype=np.int64)
        own_nodes[loc[selk]] = selk
        res = results[k]["out"].astype(np.float32)
        out[own_nodes] = res[row][:, inv_perm]
    return out


LAST_RESULTS = None


def kernel(x, W, a_src, a_dst, bias, edge_index):
    global LAST_RESULTS
    from concourse.bass_utils import run_bass_kernel_spmd

    cfg, meta, in_maps = host_prepare(x, W, a_src, a_dst, bias, edge_index)
    nc = build_program(cfg["NLOC"], cfg["NROWT"], cfg["IN_DIM"],
                       meta["Tbs"], cfg["bias_nonzero"])
    res = run_bass_kernel_spmd(
        nc, in_maps, core_ids=list(range(NCORES)),
        trace=os.environ.get("GAT_TRACE", "0") == "1")
    LAST_RESULTS = res
    return assemble_output(res.results, x.shape[0], meta["owner"],
                           meta["loc"], meta["perm_ch"])
